# revision 8
# baseline (speedup 1.0000x reference)
"""Multi-head causal self-attention on 8 Trainium2 NeuronCores.

Problem: B=4, S=2048, D=1024, H=16 heads (Dh=64), fp32, causal + key-padding
mask, out = softmax(mask(QK^T/sqrt(Dh))) V Wo^T with Q/K/V = x @ W*^T.

Sharding (data-parallel over batch x tensor-parallel over heads):
  core = 2*b + g  (b in 0..3, g in 0..1): batch b, head group g (8 heads).
  Each core computes its 8 heads' attention and a partial output projection
  through its row-slice of Wo; the host sums the two partials per batch
  (the "all-reduce" of the hint, done on host since outputs are gathered
  anyway).

Per-core kernel layout (everything f32; matmuls in float32r):
  - x^T [D, S] so projections contract D on partitions.
  - q^T, k^T stored [F=512, S] (head-major rows, 64 rows per head; heads
    2f/2f+1 live in partitions 0-63 / 64-127 of feature tile f).
  - scores computed TRANSPOSED per head: s^T[k, q] = k^T_tile.T @ q^T so the
    softmaxed tile feeds the AV matmul directly as the moving operand.
  - exp via ScalarE with fused scale (1/8) and per-key padding bias.
  - causal: only lower block-triangle computed; diagonal 128x128 blocks get a
    multiplicative triangular mask after exp; partial-width matmuls skip
    fully-masked column ranges.
  - V carries an appended ones-column per head so the AV matmul also yields
    the softmax denominators (row 64 of the [65, q] psum tile).
  - normalize: reciprocal on DVE, partition-broadcast on GpSimd, multiply on
    DVE straight into ctx^T tiles, which are the stationary operand of the
    output projection out[s, d] = ctx^T.T @ Wo_slice^T.
"""

import os
import numpy as np

import concourse.bass as bass
import concourse.mybir as mybir
import concourse.tile as tile
from concourse import bacc
from concourse.bass_utils import run_bass_kernel_spmd

P = 128
NEG = -1.0e30


def _round_f32r(a: np.ndarray) -> np.ndarray:
    """Round fp32 values to the PE's fp32r grid (11-bit mantissa,
    round-half-to-even at bit 12) so DMA-loaded tiles hold valid fp32r
    values. Matches walrus fp32_to_fp32r bit-exactly."""
    bits = np.ascontiguousarray(a, dtype=np.float32).view(np.uint32)
    low = bits & np.uint32(0xFFF)
    hi = bits & np.uint32(0xFFFFF000)
    add = (low > 0x800) | ((low == 0x800) & (((bits >> 12) & 1) == 1))
    return (hi + (add.astype(np.uint32) << 12)).view(np.float32)


class Cfg:
    def __init__(self, B=4, S=2048, D=1024, H=16, Dh=64, n_cores=8, qch=512,
                 mm_dtype="fp32r"):
        self.B, self.S, self.D, self.H, self.Dh = B, S, D, H, Dh
        self.n_cores = n_cores
        self.groups = n_cores // B              # head groups (tensor-parallel)
        self.Hc = H // self.groups              # heads per core
        self.F = self.Hc * Dh                   # per-core q/k/v feature width
        self.qch = qch                          # q columns per score matmul
        self.nqc = S // qch                     # q chunks
        self.qt_per_ch = qch // P               # 128-row q tiles per chunk
        self.nt_s = S // P                      # key/seq tiles
        self.nt_d = D // P                      # contraction tiles (D)
        self.nt_f = self.F // P                 # feature tiles
        self.heads_per_ft = P // Dh             # heads packed per feature tile
        self.mm_dtype = mm_dtype

    @property
    def mdt(self):
        return {"fp32r": mybir.dt.float32r,
                "fp32": mybir.dt.float32,
                "bf16": mybir.dt.bfloat16}[self.mm_dtype]


def build_nc(cfg: Cfg):
    f32 = mybir.dt.float32
    mdt = cfg.mdt
    S, D, F, Dh = cfg.S, cfg.D, cfg.F, cfg.Dh
    QCH = cfg.qch

    nc = bacc.Bacc("TRN2", target_bir_lowering=False, debug=False,
                   num_devices=cfg.n_cores)

    xT = nc.dram_tensor("xT", [D, S], mdt, kind="ExternalInput").ap()
    wqT = nc.dram_tensor("wqT", [D, F], mdt, kind="ExternalInput").ap()
    wkT = nc.dram_tensor("wkT", [D, F], mdt, kind="ExternalInput").ap()
    wvT = nc.dram_tensor("wvT", [D, F], mdt, kind="ExternalInput").ap()
    woT = nc.dram_tensor("woT", [F, D], mdt, kind="ExternalInput").ap()
    pbias = nc.dram_tensor("pbias", [P, cfg.nt_s], f32, kind="ExternalInput").ap()
    out = nc.dram_tensor("out", [S, D], f32, kind="ExternalOutput").ap()

    Exp = mybir.ActivationFunctionType.Exp
    mult = mybir.AluOpType.mult

    with tile.TileContext(nc) as tc:
        with (
            tc.tile_pool(name="psA", bufs=3, space="PSUM") as psA,
            tc.tile_pool(name="psB", bufs=2, space="PSUM") as psB,
            tc.tile_pool(name="psC", bufs=2, space="PSUM") as psC,
            tc.tile_pool(name="sb_qT", bufs=cfg.nt_f) as sb_qT,
            tc.tile_pool(name="sb_kT", bufs=cfg.nt_f) as sb_kT,
            tc.tile_pool(name="sb_v", bufs=cfg.nt_s) as sb_v,
            tc.tile_pool(name="sb_misc", bufs=1) as sb_misc,
        ):
            # --- constants ---
            pb = sb_misc.tile([P, cfg.nt_s], f32, tag="pbias")
            nc.sync.dma_start(pb[:], pbias)
            # triangular keep-mask in [k(part), q(free)] coords: 1 where q>=k
            tri_f = sb_misc.tile([P, P], f32, tag="tri_f")
            nc.gpsimd.memset(tri_f[:], 1.0)
            nc.gpsimd.affine_select(
                out=tri_f[:], in_=tri_f[:],
                compare_op=mybir.AluOpType.is_ge, fill=0.0,
                base=0, channel_multiplier=-1, pattern=[[1, P]],
            )
            tri = sb_misc.tile([P, P], mdt, tag="tri")
            nc.vector.tensor_copy(tri[:], tri_f[:])
            ones_c = sb_misc.tile([P, 1], f32, tag="ones_c")
            nc.gpsimd.memset(ones_c[:], 1.0)

            qT_t = [sb_qT.tile([P, S], mdt, tag="qT", name="qT") for _ in range(cfg.nt_f)]
            kT_t = [sb_kT.tile([P, S], mdt, tag="kT", name="kT") for _ in range(cfg.nt_f)]
            v_t = [sb_v.tile([P, cfg.Hc * (Dh + 1)], mdt, tag="v", name="v") for _ in range(cfg.nt_s)]

            # ---------------- Phase 1: Q/K/V projections ----------------
            with (
                tc.tile_pool(name="sb_xt", bufs=cfg.nt_d) as sb_xt,
                tc.tile_pool(name="sb_w", bufs=2 * cfg.nt_d) as sb_w,
            ):
                xt = []
                for d in range(cfg.nt_d):
                    t = sb_xt.tile([P, S], mdt, tag="xt")
                    nc.sync.dma_start(t[:], xT[d * P:(d + 1) * P, :])
                    xt.append(t)

                # q^T / k^T: [F, S] = w*T.T @ x^T
                for wdram, dstT in ((wqT, qT_t), (wkT, kT_t)):
                    wt = []
                    for d in range(cfg.nt_d):
                        t = sb_w.tile([P, F], mdt, tag="w")
                        nc.sync.dma_start(t[:], wdram[d * P:(d + 1) * P, :])
                        wt.append(t)
                    for m in range(cfg.nt_f):
                        for c in range(S // 512):
                            ps = psA.tile([P, 512], f32, tag="psA", name="ps")
                            for d in range(cfg.nt_d):
                                nc.tensor.matmul(
                                    ps[:],
                                    wt[d][:, m * P:(m + 1) * P],
                                    xt[d][:, c * 512:(c + 1) * 512],
                                    start=(d == 0), stop=(d == cfg.nt_d - 1),
                                )
                            nc.vector.tensor_copy(
                                dstT[m][:, c * 512:(c + 1) * 512], ps[:])

                # v natural: [S, F] = x^T.T @ wvT, stored with an appended
                # ones column per head ([64 features | 1] x Hc).
                wv = []
                for d in range(cfg.nt_d):
                    t = sb_w.tile([P, F], mdt, tag="w")
                    nc.sync.dma_start(t[:], wvT[d * P:(d + 1) * P, :])
                    wv.append(t)
                for st in range(cfg.nt_s):
                    ps = psA.tile([P, F], f32, tag="psA", name="ps")
                    for d in range(cfg.nt_d):
                        nc.tensor.matmul(
                            ps[:],
                            xt[d][:, st * P:(st + 1) * P],
                            wv[d][:],
                            start=(d == 0), stop=(d == cfg.nt_d - 1),
                        )
                    dst = v_t[st][:].rearrange("p (h e) -> p h e", e=Dh + 1)
                    nc.vector.tensor_copy(
                        dst[:, :, 0:Dh],
                        ps[:].rearrange("p (h e) -> p h e", e=Dh),
                    )
                    nc.vector.tensor_copy(
                        dst[:, :, Dh:Dh + 1],
                        ones_c[:, None, 0:1].to_broadcast([P, cfg.Hc, 1]))

            # ---------------- Phase 2+3: attention + output proj ----------
            with (
                tc.tile_pool(name="sb_ctx", bufs=cfg.nt_f) as sb_ctx,
                tc.tile_pool(name="sb_wo", bufs=cfg.nt_f) as sb_wo,
                tc.tile_pool(name="sb_exp", bufs=8) as sb_exp,
                tc.tile_pool(name="sb_out", bufs=3) as sb_out,
                tc.tile_pool(name="sb_rc", bufs=4) as sb_rc,
            ):
                ctxT_t = [sb_ctx.tile([P, S], mdt, tag="ctxT", name="ctxT") for _ in range(cfg.nt_f)]
                wo_t = []
                for f in range(cfg.nt_f):
                    t = sb_wo.tile([P, D], mdt, tag="wo")
                    nc.sync.dma_start(t[:], woT[f * P:(f + 1) * P, :])
                    wo_t.append(t)

                for c in range(cfg.nqc):
                    ktiles = cfg.qt_per_ch * (c + 1)
                    for h in range(cfg.Hc):
                        f, r = divmod(h, cfg.heads_per_ft)
                        rows = slice(r * Dh, (r + 1) * Dh)
                        pav = psB.tile([Dh + 1, QCH], f32, tag="pav")
                        for t in range(ktiles):
                            j = t - cfg.qt_per_ch * c
                            col0 = max(0, j * P)
                            pss = psA.tile([P, QCH], f32, tag="psA", name="pss")
                            nc.tensor.matmul(
                                pss[:, col0:],
                                kT_t[f][rows, t * P:(t + 1) * P],
                                qT_t[f][rows, c * QCH + col0:(c + 1) * QCH],
                                start=True, stop=True,
                                tile_position=(r * Dh, 0),
                            )
                            et = sb_exp.tile([P, QCH], mdt, tag="exp")
                            nc.scalar.activation(
                                et[:, col0:], pss[:, col0:], Exp,
                                bias=pb[:, t:t + 1], scale=float(Dh) ** -0.5,
                            )
                            if j >= 0:
                                nc.vector.tensor_tensor(
                                    et[:, col0:col0 + P],
                                    et[:, col0:col0 + P], tri[:], mult)
                            nc.tensor.matmul(
                                pav[:, col0:],
                                v_t[t][:, h * (Dh + 1):(h + 1) * (Dh + 1)],
                                et[:, col0:],
                                start=(t == 0), stop=(t == ktiles - 1),
                            )
                        rc = sb_rc.tile([1, QCH], f32, tag="rc")
                        rcb = sb_rc.tile([Dh, QCH], f32, tag="rcb")
                        nc.vector.reciprocal(rc[:], pav[Dh:Dh + 1, :])
                        nc.gpsimd.partition_broadcast(rcb[:], rc[:])
                        nc.vector.tensor_tensor(
                            ctxT_t[f][rows, c * QCH:(c + 1) * QCH],
                            pav[0:Dh, :], rcb[:], mult)

                    # output projection for this chunk's rows
                    for u in range(cfg.qt_per_ch):
                        st = c * cfg.qt_per_ch + u
                        ot = sb_out.tile([P, D], f32, tag="ot")
                        dw = min(512, D)
                        for dch in range(D // dw):
                            pwo = psC.tile([P, dw], f32, tag="pwo")
                            for f2 in range(cfg.nt_f):
                                nc.tensor.matmul(
                                    pwo[:],
                                    ctxT_t[f2][:, st * P:(st + 1) * P],
                                    wo_t[f2][:, dch * dw:(dch + 1) * dw],
                                    start=(f2 == 0), stop=(f2 == cfg.nt_f - 1),
                                )
                            nc.vector.tensor_copy(
                                ot[:, dch * dw:(dch + 1) * dw], pwo[:])
                        nc.sync.dma_start(out[st * P:(st + 1) * P, :], ot[:])

    nc.compile()
    return nc


_NC_CACHE = {}


def _get_nc(cfg: Cfg):
    key = (cfg.B, cfg.S, cfg.D, cfg.H, cfg.n_cores, cfg.qch, cfg.mm_dtype)
    if key not in _NC_CACHE:
        _NC_CACHE[key] = build_nc(cfg)
    return _NC_CACHE[key]


def make_in_maps(cfg: Cfg, x_self, padding_mask, Wq, Wk, Wv, Wo):
    """Host-side sharding: slice + transpose per core."""
    rnd = _round_f32r if cfg.mm_dtype == "fp32r" else (
        lambda a: np.ascontiguousarray(a, dtype=np.float32))
    in_maps = []
    for core in range(cfg.n_cores):
        b, g = divmod(core, cfg.groups)
        fsl = slice(g * cfg.F, (g + 1) * cfg.F)
        pbias = np.where(padding_mask[b], np.float32(NEG), np.float32(0.0))
        in_maps.append({
            "xT": rnd(x_self[b].T),
            "wqT": rnd(Wq[fsl, :].T),
            "wkT": rnd(Wk[fsl, :].T),
            "wvT": rnd(Wv[fsl, :].T),
            "woT": rnd(Wo[:, fsl].T),
            "pbias": np.ascontiguousarray(
                pbias.reshape(cfg.nt_s, P).T).astype(np.float32),
        })
    return in_maps


def kernel(x_self, x_other, padding_mask, Wq, Wk, Wv, Wo, _trace=False):
    x_self = np.asarray(x_self, dtype=np.float32)
    padding_mask = np.asarray(padding_mask)
    Wq = np.asarray(Wq, dtype=np.float32)
    Wk = np.asarray(Wk, dtype=np.float32)
    Wv = np.asarray(Wv, dtype=np.float32)
    Wo = np.asarray(Wo, dtype=np.float32)

    B, S, D = x_self.shape
    cfg = Cfg(B=B, S=S, D=D)
    nc = _get_nc(cfg)
    in_maps = make_in_maps(cfg, x_self, padding_mask, Wq, Wk, Wv, Wo)
    res = run_bass_kernel_spmd(
        nc, in_maps, core_ids=list(range(cfg.n_cores)), trace=_trace)

    out = np.zeros((B, S, D), dtype=np.float32)
    for core in range(cfg.n_cores):
        b = core // cfg.groups
        out[b] += res.results[core]["out"]
    if _trace:
        kernel.last_exec_time_ns = res.exec_time_ns
        kernel.last_results = res
    return out
